# revision 16
# baseline (speedup 1.0000x reference)
"""SANet-style attention (nn_Attention_1382979470038) on 8 TRN2 NeuronCores.

Sharding: 8 cores = 4 batches x 2 content-token halves (sequence parallel on
N, style tokens replicated within each pair).  No collectives: each core
computes output columns [C=512, N_loc=2048] of its batch independently.

Factored score (softmax over m makes per-n-constant terms cancel):
  S[n,m] = F[:,n].G[:,m]  ==softmax==  FB[:,n].xs[:,m]
  where FB = diag(rstd_s) B^T diag(rstd_c) xc + b_FB      [C, N_loc]
        B  = f_w^T g_w   (host),  b_FB = rstd_s*(g_w^T f_b - B_rs^T mean_c)
  so the G conv disappears entirely and RAW style (held in SBUF) is the
  score stationary.  Per-core tensor work:
    Ht  = style^T (out_w h_w)^T          [M, C]    128 mm
    FB  conv                              [C, N]     64 mm
    St  = style^T FB                      [M, N]    512 mm
    P   = exp(St)  (scalar engine)
    den = ones^T V, V = sum_mt P[mt]  (V on vector engine -> 1 mm/ch)
    out = (Ht^T P) * (1/den) + ob2        [C, N]    512 mm
  Instance-norm stats via one-pass DVE bn_stats/bn_aggr (no scalar Square).
"""

import sys

sys.path.insert(0, "/opt/trn_rl_repo")

import numpy as np

import concourse.bass as bass
import concourse.tile as tile
from concourse import mybir

P = 128
C = 512
HW = 4096
NLOC = 2048
EPS = 1e-5
KT = C // P        # 4 k-tiles of 128 channels
NCH = NLOC // 512  # 4 n-chunks of 512
MCH = HW // 512    # 8 m-chunks of 512
MT = HW // P       # 32 m-tiles of 128

F32 = mybir.dt.float32
BF16 = mybir.dt.bfloat16
F16 = mybir.dt.float16

AF = mybir.ActivationFunctionType
ALU = mybir.AluOpType


def build_nc(hoist=True):
    nc = bass.Bass()
    # k-major layouts: [128, KT * cols]; column block k holds rows k*128..
    cAp = nc.declare_dram_parameter("cA", [P, KT * NLOC], F16, isOutput=False)
    cbbfp = nc.declare_dram_parameter("cB_bf", [P, KT * NLOC], F16,
                                      isOutput=False)
    sbfp = nc.declare_dram_parameter("style_bf", [P, KT * HW], F16,
                                     isOutput=False)
    hwbfp = nc.declare_dram_parameter("hw_bf", [P, KT * C], F16,
                                      isOutput=False)
    bmatp = nc.declare_dram_parameter("bmat", [P, KT * C], F16,
                                      isOutput=False)
    pbp = nc.declare_dram_parameter("pbias", [P, 2 * KT], F32, isOutput=False)
    out = nc.declare_dram_parameter("out", [C, NLOC], F32, isOutput=True)

    with tile.TileContext(nc) as tc:
        _build(tc, cAp, cbbfp, sbfp, hwbfp, bmatp, pbp, out)
    if hoist:
        _hoist_excess_waits(nc)
    return nc


# Walrus caps sync-wait commands per instruction (Activation/TensorScalar fit
# only one).  Hoist excess waits onto injected same-engine NOPs placed just
# before the instruction: engines execute in order, so semantics match.
def _hoist_excess_waits(nc):
    ctr = [0]

    def mknop(engine, debug, waits, updates):
        ctr[0] += 1
        return mybir.InstNoOp(
            name=f"WH-{ctr[0]}", opcode="NoOp", engine=engine, debug=debug,
            ins=[], outs=[],
            sync_info=mybir.SyncInfo(on_wait=waits, on_update=updates),
        )

    for fn in nc.m.functions:
        for blk in fn.blocks:
            newl = []
            changed = False
            for inst in blk.instructions:
                si = getattr(inst, "sync_info", None)
                if si is not None and si.on_wait and len(si.on_wait) > 1:
                    waits = list(si.on_wait)
                    keep, hoist = waits[-1:], waits[:-1]
                    eng = getattr(inst, "engine", None)
                    for w in hoist:
                        newl.append(mknop(eng, inst.debug, [w], []))
                    inst.sync_info = mybir.SyncInfo(
                        on_wait=keep, on_update=list(si.on_update))
                    changed = True
                newl.append(inst)
            if changed:
                blk.instructions = newl


def _build(tc, cAp, cbbfp, sbfp, hwbfp, bmatp, pbp, out):
    nc = tc.nc
    from contextlib import ExitStack

    ctx = ExitStack()
    with ctx:
        # ---------- long-lived pools ----------
        cons = ctx.enter_context(tc.tile_pool(name="cons", bufs=1))
        stylep = ctx.enter_context(tc.tile_pool(name="stylep", bufs=1))
        htp = ctx.enter_context(tc.tile_pool(name="htp", bufs=1))
        fpool = ctx.enter_context(tc.tile_pool(name="fpool", bufs=4))
        statw = ctx.enter_context(tc.tile_pool(name="statw", bufs=1))
        s1 = ctx.enter_context(tc.tile_pool(name="s1", bufs=64))
        # PSUM pools (8 banks: 4 + 3 + 1)
        stps = ctx.enter_context(tc.tile_pool(name="stps", bufs=3,
                                              space="PSUM"))
        mmps = ctx.enter_context(tc.tile_pool(name="mmps", bufs=3,
                                              space="PSUM"))

        style_sb = stylep.tile([P, MCH, KT, 512], F16, tag="style")
        Ht_sb = htp.tile([P, MT, C], BF16, tag="Ht")
        hw_sb = cons.tile([P, KT, C], F16, tag="hw_sb")
        pb_sb = cons.tile([P, 2 * KT], F32, tag="pb_sb")
        ones_bf = cons.tile([P, P], BF16, tag="ones_bf")
        F_raw = [fpool.tile([P, NLOC], F16, tag="Fr", name=f"Fr{k}")
                 for k in range(KT)]
        F_sb = [fpool.tile([P, NLOC], F16, tag="F", name=f"F{k}")
                for k in range(KT)]

        with tc.tile_pool(name="contp", bufs=1) as contp, \
             tc.tile_pool(name="bp", bufs=1) as bp:
            cA_s = contp.tile([P, KT, NCH, 512], F16, tag="cA")
            cB_s = contp.tile([P, KT, NCH, 512], F16, tag="cB")
            bmat_sb = bp.tile([P, KT, C], F16, tag="bmat")
            B_rs = bp.tile([P, KT, C], F16, tag="Brs")

            # ---------- DMAs, priority order (style chunks pace the Ht
            # conv; content leapfrogs the late style chunks so stats can
            # start early without starving the conv) ---------------------
            nc.sync.dma_start(style_sb[:, 0, 0:2, :], sbfp[:, 0:1024])
            nc.sync.dma_start(hw_sb[:, 0:2, :], hwbfp[:, 0:2 * C])
            nc.sync.dma_start(style_sb[:, 0, 2:4, :], sbfp[:, 1024:2048])
            nc.sync.dma_start(hw_sb[:, 2:4, :], hwbfp[:, 2 * C:4 * C])
            def style_chunk(ch):
                nc.sync.dma_start(
                    style_sb[:, ch, :, :],
                    sbfp[:, ch * KT * 512:(ch + 1) * KT * 512])

            def cont_piece(dst, srcp, k):
                nc.sync.dma_start(dst[:, k, :, :],
                                  srcp[:, k * NLOC:(k + 1) * NLOC])

            style_chunk(1)
            cont_piece(cA_s, cAp, 0)
            style_chunk(2)
            cont_piece(cA_s, cAp, 1)
            style_chunk(3)
            cont_piece(cA_s, cAp, 2)
            style_chunk(4)
            cont_piece(cA_s, cAp, 3)
            style_chunk(5)
            cont_piece(cB_s, cbbfp, 0)
            style_chunk(6)
            cont_piece(cB_s, cbbfp, 1)
            cont_piece(cB_s, cbbfp, 2)
            style_chunk(7)
            cont_piece(cB_s, cbbfp, 3)
            nc.sync.dma_start(pb_sb[:], pbp[:, :])
            nc.sync.dma_start(bmat_sb[:, :, :], bmatp[:, :])
            nc.vector.memset(ones_bf[:], 1.0)

            # ---------- phase A: Ht conv + stats ------------------------
            # All Ht psum->SBUF copies on scalar; all bn_stats on vector,
            # hand-ordered so the content-stat chain (which gates the FB
            # conv) completes right as the Ht conv ends, and the style
            # chain (which only gates the post-FB fixup) fills the tail.
            sstats = [statw.tile([P, MCH, 6], F32, tag=f"sst{k}",
                                 name=f"sst{k}") for k in range(KT)]
            cstats = [statw.tile([P, 2 * NCH, 6], F32, tag=f"cst{k}",
                                 name=f"cst{k}") for k in range(KT)]
            cmean_h, crstd, srstd = [], [], []
            c_varp, c_std, s_varp, s_std = [], [], [], []
            for ch in range(MCH):
                if ch < 4:
                    for k in range(KT):
                        nc.vector.bn_stats(sstats[k][:, ch, :],
                                           style_sb[:, ch, k, :])
                for mi in range(4):
                    mt = ch * 4 + mi
                    ps = stps.tile([P, C], F32, tag="stps")
                    for k in range(KT):
                        nc.tensor.matmul(
                            ps[:],
                            style_sb[:, ch, k, mi * P:(mi + 1) * P],
                            hw_sb[:, k, :],
                            start=(k == 0), stop=(k == KT - 1))
                    nc.scalar.activation(Ht_sb[:, mt, :], ps[:], AF.Copy)
                if ch == 3:
                    for cc in range(NCH):
                        for k in range(KT):
                            nc.vector.bn_stats(cstats[k][:, cc, :],
                                               cA_s[:, k, cc, :])
                if ch == 5:
                    # k-outer so each k's aggregate chain starts as soon as
                    # its four cB chunks are done (pipelines with bn tail)
                    for k in range(KT):
                        for cc in range(NCH):
                            nc.vector.bn_stats(cstats[k][:, NCH + cc, :],
                                               cB_s[:, k, cc, :])
                        mv = s1.tile([P, 2], F32, tag="s2")
                        nc.vector.bn_aggr(mv[:], cstats[k][:, :, :])
                        mh = s1.tile([P, 1], F16, tag="s1h")
                        nc.vector.tensor_copy(mh[:], mv[:, 0:1])
                        cmean_h.append(mh)
                        vp_ = s1.tile([P, 1], F32, tag="s1")
                        nc.vector.tensor_scalar(vp_[:], mv[:, 1:2],
                                                HW / (HW - 1.0), EPS,
                                                op0=ALU.mult, op1=ALU.add)
                        c_varp.append(vp_)
                        st = s1.tile([P, 1], F32, tag="s1")
                        nc.scalar.activation(st[:], vp_[:], AF.Sqrt)
                        c_std.append(st)
                    for k in range(KT):
                        r = s1.tile([P, 1], F32, tag="s1")
                        nc.vector.reciprocal(r[:], c_std[k][:])
                        crstd.append(r)
                        nc.vector.tensor_scalar_mul(
                            B_rs[:, k, :], bmat_sb[:, k, :], r[:])

            # ---------- FB bias fixup mms + FB conv (raw; style-side
            # scale/bias applied later on vector once srstd lands) -------
            ubias = []
            for j in range(KT):
                psb = mmps.tile([P, 1], F32, tag="ups", name=f"ups{j}",
                                bufs=2)
                for k in range(KT):
                    nc.tensor.matmul(
                        psb[:], B_rs[:, k, j * P:(j + 1) * P], cmean_h[k][:],
                        start=(k == 0), stop=(k == KT - 1))
                bp_ = s1.tile([P, 1], F32, tag="s1")
                nc.vector.tensor_sub(bp_[:], pb_sb[:, KT + j:KT + j + 1],
                                     psb[:])
                ubias.append(bp_)
            for ch in range(NCH):
                for j in range(KT):
                    ps = stps.tile([P, 512], F32, tag="stps")
                    for k in range(KT):
                        nc.tensor.matmul(
                            ps[:], B_rs[:, k, j * P:(j + 1) * P],
                            cA_s[:, k, ch, :],
                            start=(k == 0), stop=(k == KT - 1))
                    nc.scalar.activation(
                        F_raw[j][:, ch * 512:(ch + 1) * 512], ps[:],
                        AF.Copy)
                if ch == 1:
                    # style stats tail on vector
                    for sch in range(4, MCH):
                        for k in range(KT):
                            nc.vector.bn_stats(sstats[k][:, sch, :],
                                               style_sb[:, sch, k, :])
                    for k in range(KT):
                        mv = s1.tile([P, 2], F32, tag="s2")
                        nc.vector.bn_aggr(mv[:], sstats[k][:, :, :])
                        vp_ = s1.tile([P, 1], F32, tag="s1")
                        nc.vector.tensor_scalar(vp_[:], mv[:, 1:2],
                                                HW / (HW - 1.0), EPS,
                                                op0=ALU.mult, op1=ALU.add)
                        s_varp.append(vp_)
                if ch == 2:
                    for k in range(KT):
                        st = s1.tile([P, 1], F32, tag="s1")
                        nc.scalar.activation(st[:], s_varp[k][:], AF.Sqrt)
                        s_std.append(st)
            for k in range(KT):
                r = s1.tile([P, 1], F32, tag="s1")
                nc.vector.reciprocal(r[:], s_std[k][:])
                srstd.append(r)
            bias_t = []
            for j in range(KT):
                bt = s1.tile([P, 1], F32, tag="s1")
                nc.vector.tensor_mul(bt[:], ubias[j][:], srstd[j][:])
                bias_t.append(bt)
            # FB = F_raw * srstd + bias  (vector, ch0 first: gates scores)
            for ch in range(NCH):
                for j in range(KT):
                    nc.vector.tensor_scalar(
                        F_sb[j][:, ch * 512:(ch + 1) * 512],
                        F_raw[j][:, ch * 512:(ch + 1) * 512],
                        srstd[j][:], bias_t[j][:],
                        op0=ALU.mult, op1=ALU.add)

        # ---------- phase B: attention ----------
        with tc.tile_pool(name="expp", bufs=2) as expp, \
             tc.tile_pool(name="vp", bufs=2) as vp, \
             tc.tile_pool(name="styp", bufs=3) as styp, \
             tc.tile_pool(name="outp", bufs=3) as outp:
            for ch in range(NCH):
                exp_t = expp.tile([P, MT, 512], BF16, tag="exp")
                V = vp.tile([P, 512], F32, tag="V")
                V_bf = vp.tile([P, 512], BF16, tag="Vbf")
                rden = vp.tile([P, 512], F32, tag="rden")
                for mt in range(MT):
                    ps = stps.tile([P, 512], F32, tag="stps")
                    for k in range(KT):
                        nc.tensor.matmul(
                            ps[:],
                            style_sb[:, mt // 4, k,
                                     (mt % 4) * P:(mt % 4 + 1) * P],
                            F_sb[k][:, ch * 512:(ch + 1) * 512],
                            start=(k == 0), stop=(k == KT - 1))
                    nc.scalar.activation(exp_t[:, mt, :], ps[:], AF.Exp)
                    if mt == 0:
                        nc.vector.tensor_copy(V[:], exp_t[:, 0, :])
                    elif mt == MT - 1:
                        nc.vector.tensor_add(V_bf[:], V[:], exp_t[:, mt, :])
                    else:
                        nc.vector.tensor_add(V[:], V[:], exp_t[:, mt, :])

                apsums = []
                for j in range(KT):
                    ps = mmps.tile([P, 512], F32, tag="mmps")
                    for mt in range(MT):
                        nc.tensor.matmul(
                            ps[:], Ht_sb[:, mt, j * P:(j + 1) * P],
                            exp_t[:, mt, :],
                            start=(mt == 0), stop=(mt == MT - 1))
                    apsums.append(ps)
                    if j == 0:
                        dps = mmps.tile([P, 512], F32, tag="mmps")
                        nc.tensor.matmul(dps[:], ones_bf[:], V_bf[:],
                                         start=True, stop=True)
                        nc.vector.reciprocal(rden[:], dps[:])
                    s_t = styp.tile([P, 512], F32, tag="sty")
                    nc.vector.tensor_mul(s_t[:], ps[:], rden[:])
                    o_t = outp.tile([P, 512], F32, tag="outsb")
                    nc.scalar.activation(o_t[:], s_t[:], AF.Identity,
                                         bias=pb_sb[:, j:j + 1])
                    nc.sync.dma_start(
                        out[j * P:(j + 1) * P, ch * 512:(ch + 1) * 512],
                        o_t[:])


def _kmajor(x, cols):
    """[KT*128, cols] -> [128, KT*cols] with column block k = rows k*128.."""
    return np.ascontiguousarray(
        np.asarray(x).reshape(KT, P, cols).transpose(1, 0, 2)
        .reshape(P, KT * cols), dtype=np.float32)


_NC_CACHE = None


def _get_nc():
    global _NC_CACHE
    if _NC_CACHE is None:
        _NC_CACHE = build_nc()
    return _NC_CACHE


def make_in_maps(content, style, f_w, f_b, g_w, g_b, h_w, h_b, out_w, out_b):
    b, Cc, H, W = content.shape
    hw = H * W
    cf = np.ascontiguousarray(content.reshape(b, Cc, hw), dtype=np.float32)
    sf = np.ascontiguousarray(style.reshape(b, Cc, hw), dtype=np.float32)
    ob2 = (np.asarray(out_b, np.float64)
           + np.asarray(out_w, np.float64) @ np.asarray(h_b, np.float64))
    walpha = (np.asarray(g_w, np.float64).T @ np.asarray(f_b, np.float64))
    pbias = np.concatenate([
        ob2.astype(np.float32).reshape(KT, P).T,
        walpha.astype(np.float32).reshape(KT, P).T], axis=1)
    hw2 = np.asarray(out_w, np.float64) @ np.asarray(h_w, np.float64)
    bmat = (np.asarray(f_w, np.float64).T @ np.asarray(g_w, np.float64))
    wT = {
        "hw_bf": _kmajor(hw2.T.astype(np.float32), C).astype(np.float16),
        "bmat": _kmajor(bmat.astype(np.float32), C).astype(np.float16),
        "pbias": np.ascontiguousarray(pbias, dtype=np.float32),
    }
    in_maps = []
    for core in range(8):
        bi, hi = core // 2, core % 2
        in_maps.append({
            "cA": _kmajor(cf[bi][:, hi * NLOC:(hi + 1) * NLOC],
                          NLOC).astype(np.float16),
            "cB_bf": _kmajor(cf[bi][:, (1 - hi) * NLOC:(2 - hi) * NLOC],
                             NLOC).astype(np.float16),
            "style_bf": np.concatenate(
                [_kmajor(sf[bi][:, ch * 512:(ch + 1) * 512], 512)
                 for ch in range(MCH)], axis=1).astype(np.float16),
            **wT,
        })
    return in_maps


def kernel(content, style, f_w, f_b, g_w, g_b, h_w, h_b, out_w, out_b):
    from concourse.bass_utils import run_bass_kernel_spmd

    global _LAST_IN_MAPS
    in_maps = make_in_maps(content, style, f_w, f_b, g_w, g_b, h_w, h_b,
                           out_w, out_b)
    _LAST_IN_MAPS = in_maps
    b, Cc, H, W = content.shape
    hw = H * W
    nc = _get_nc()
    res = run_bass_kernel_spmd(nc, in_maps, core_ids=list(range(8)))
    outf = np.empty((b, Cc, hw), dtype=np.float32)
    for core in range(8):
        bi, hi = core // 2, core % 2
        outf[bi][:, hi * NLOC:(hi + 1) * NLOC] = res.results[core]["out"]
    return outf.reshape(b, Cc, H, W)


# revision 17
# speedup vs baseline: 1.0015x; 1.0015x over previous
"""SANet-style attention (nn_Attention_1382979470038) on 8 TRN2 NeuronCores.

Sharding: 8 cores = 4 batches x 2 content-token halves (sequence parallel on
N, style tokens replicated within each pair).  No collectives: each core
computes output columns [C=512, N_loc=2048] of its batch independently.

Factored score (softmax over m makes per-n-constant terms cancel):
  S[n,m] = F[:,n].G[:,m]  ==softmax==  FB[:,n].xs[:,m]
  where FB = diag(rstd_s) B^T diag(rstd_c) xc + b_FB      [C, N_loc]
        B  = f_w^T g_w   (host),  b_FB = rstd_s*(g_w^T f_b - B_rs^T mean_c)
  so the G conv disappears entirely and RAW style (held in SBUF) is the
  score stationary.  Per-core tensor work:
    Ht  = style^T (out_w h_w)^T          [M, C]    128 mm
    FB  conv                              [C, N]     64 mm
    St  = style^T FB                      [M, N]    512 mm
    P   = exp(St)  (scalar engine)
    den = ones^T V, V = sum_mt P[mt]  (V on vector engine -> 1 mm/ch)
    out = (Ht^T P) * (1/den) + ob2        [C, N]    512 mm
  Instance-norm stats via one-pass DVE bn_stats/bn_aggr (no scalar Square).
"""

import sys

sys.path.insert(0, "/opt/trn_rl_repo")

import numpy as np

import concourse.bass as bass
import concourse.tile as tile
from concourse import mybir

P = 128
C = 512
HW = 4096
NLOC = 2048
EPS = 1e-5
KT = C // P        # 4 k-tiles of 128 channels
NCH = NLOC // 512  # 4 n-chunks of 512
MCH = HW // 512    # 8 m-chunks of 512
MT = HW // P       # 32 m-tiles of 128

F32 = mybir.dt.float32
BF16 = mybir.dt.bfloat16
F16 = mybir.dt.float16

AF = mybir.ActivationFunctionType
ALU = mybir.AluOpType


def build_nc(hoist=True):
    nc = bass.Bass()
    # k-major layouts: [128, KT * cols]; column block k holds rows k*128..
    cAp = nc.declare_dram_parameter("cA", [P, KT * NLOC], F16, isOutput=False)
    cbbfp = nc.declare_dram_parameter("cB_bf", [P, KT * NLOC], F16,
                                      isOutput=False)
    sbfp = nc.declare_dram_parameter("style_bf", [P, KT * HW], F16,
                                     isOutput=False)
    hwbfp = nc.declare_dram_parameter("hw_bf", [P, KT * C], F16,
                                      isOutput=False)
    bmatp = nc.declare_dram_parameter("bmat", [P, KT * C], F16,
                                      isOutput=False)
    pbp = nc.declare_dram_parameter("pbias", [P, 2 * KT], F32, isOutput=False)
    out = nc.declare_dram_parameter("out", [C, NLOC], F32, isOutput=True)

    with tile.TileContext(nc) as tc:
        _build(tc, cAp, cbbfp, sbfp, hwbfp, bmatp, pbp, out)
    if hoist:
        _hoist_excess_waits(nc)
    return nc


# Walrus caps sync-wait commands per instruction (Activation/TensorScalar fit
# only one).  Hoist excess waits onto injected same-engine NOPs placed just
# before the instruction: engines execute in order, so semantics match.
def _hoist_excess_waits(nc):
    ctr = [0]

    def mknop(engine, debug, waits, updates):
        ctr[0] += 1
        return mybir.InstNoOp(
            name=f"WH-{ctr[0]}", opcode="NoOp", engine=engine, debug=debug,
            ins=[], outs=[],
            sync_info=mybir.SyncInfo(on_wait=waits, on_update=updates),
        )

    for fn in nc.m.functions:
        for blk in fn.blocks:
            newl = []
            changed = False
            for inst in blk.instructions:
                si = getattr(inst, "sync_info", None)
                if si is not None and si.on_wait and len(si.on_wait) > 1:
                    waits = list(si.on_wait)
                    keep, hoist = waits[-1:], waits[:-1]
                    eng = getattr(inst, "engine", None)
                    for w in hoist:
                        newl.append(mknop(eng, inst.debug, [w], []))
                    inst.sync_info = mybir.SyncInfo(
                        on_wait=keep, on_update=list(si.on_update))
                    changed = True
                newl.append(inst)
            if changed:
                blk.instructions = newl


def _build(tc, cAp, cbbfp, sbfp, hwbfp, bmatp, pbp, out):
    nc = tc.nc
    from contextlib import ExitStack

    ctx = ExitStack()
    with ctx:
        # ---------- long-lived pools ----------
        cons = ctx.enter_context(tc.tile_pool(name="cons", bufs=1))
        stylep = ctx.enter_context(tc.tile_pool(name="stylep", bufs=1))
        htp = ctx.enter_context(tc.tile_pool(name="htp", bufs=1))
        fpool = ctx.enter_context(tc.tile_pool(name="fpool", bufs=4))
        statw = ctx.enter_context(tc.tile_pool(name="statw", bufs=1))
        s1 = ctx.enter_context(tc.tile_pool(name="s1", bufs=64))
        # PSUM pools (8 banks: 4 + 3 + 1)
        stps = ctx.enter_context(tc.tile_pool(name="stps", bufs=3,
                                              space="PSUM"))
        mmps = ctx.enter_context(tc.tile_pool(name="mmps", bufs=3,
                                              space="PSUM"))

        style_sb = stylep.tile([P, MCH, KT, 512], F16, tag="style")
        Ht_sb = htp.tile([P, MT, C], BF16, tag="Ht")
        hw_sb = cons.tile([P, KT, C], F16, tag="hw_sb")
        pb_sb = cons.tile([P, 2 * KT], F32, tag="pb_sb")
        ones_bf = cons.tile([P, P], BF16, tag="ones_bf")
        F_raw = [fpool.tile([P, NLOC], F16, tag="Fr", name=f"Fr{k}")
                 for k in range(KT)]
        F_sb = [fpool.tile([P, NLOC], F16, tag="F", name=f"F{k}")
                for k in range(KT)]

        with tc.tile_pool(name="contp", bufs=1) as contp, \
             tc.tile_pool(name="bp", bufs=1) as bp:
            cA_s = contp.tile([P, KT, NCH, 512], F16, tag="cA")
            cB_s = contp.tile([P, KT, NCH, 512], F16, tag="cB")
            bmat_sb = bp.tile([P, KT, C], F16, tag="bmat")
            B_rs = bp.tile([P, KT, C], F16, tag="Brs")

            # ---------- DMAs, priority order (style chunks pace the Ht
            # conv; content leapfrogs the late style chunks so stats can
            # start early without starving the conv) ---------------------
            nc.sync.dma_start(style_sb[:, 0, 0:2, :], sbfp[:, 0:1024])
            nc.sync.dma_start(hw_sb[:, 0:2, :], hwbfp[:, 0:2 * C])
            nc.sync.dma_start(style_sb[:, 0, 2:4, :], sbfp[:, 1024:2048])
            nc.sync.dma_start(hw_sb[:, 2:4, :], hwbfp[:, 2 * C:4 * C])
            def style_chunk(ch):
                nc.sync.dma_start(
                    style_sb[:, ch, :, :],
                    sbfp[:, ch * KT * 512:(ch + 1) * KT * 512])

            def cont_piece(dst, srcp, k):
                nc.sync.dma_start(dst[:, k, :, :],
                                  srcp[:, k * NLOC:(k + 1) * NLOC])

            style_chunk(1)
            cont_piece(cA_s, cAp, 0)
            style_chunk(2)
            cont_piece(cA_s, cAp, 1)
            style_chunk(3)
            cont_piece(cA_s, cAp, 2)
            style_chunk(4)
            cont_piece(cA_s, cAp, 3)
            style_chunk(5)
            cont_piece(cB_s, cbbfp, 0)
            style_chunk(6)
            cont_piece(cB_s, cbbfp, 1)
            cont_piece(cB_s, cbbfp, 2)
            style_chunk(7)
            cont_piece(cB_s, cbbfp, 3)
            nc.sync.dma_start(pb_sb[:], pbp[:, :])
            nc.sync.dma_start(bmat_sb[:, :, :], bmatp[:, :])
            nc.vector.memset(ones_bf[:], 1.0)

            # ---------- phase A: Ht conv + stats ------------------------
            # All Ht psum->SBUF copies on scalar; all bn_stats on vector,
            # hand-ordered so the content-stat chain (which gates the FB
            # conv) completes right as the Ht conv ends, and the style
            # chain (which only gates the post-FB fixup) fills the tail.
            sstats = [statw.tile([P, MCH, 6], F32, tag=f"sst{k}",
                                 name=f"sst{k}") for k in range(KT)]
            cstats = [statw.tile([P, 2 * NCH, 6], F32, tag=f"cst{k}",
                                 name=f"cst{k}") for k in range(KT)]
            cmean_h, crstd, srstd = [], [], []
            c_varp, c_std, s_varp, s_std = [], [], [], []
            for ch in range(MCH):
                if ch < 4:
                    for k in range(KT):
                        nc.vector.bn_stats(sstats[k][:, ch, :],
                                           style_sb[:, ch, k, :])
                for mi in range(4):
                    mt = ch * 4 + mi
                    ps = stps.tile([P, C], F32, tag="stps")
                    for k in range(KT):
                        nc.tensor.matmul(
                            ps[:],
                            style_sb[:, ch, k, mi * P:(mi + 1) * P],
                            hw_sb[:, k, :],
                            start=(k == 0), stop=(k == KT - 1))
                    nc.scalar.activation(Ht_sb[:, mt, :], ps[:], AF.Copy)
                if ch == 3:
                    for cc in range(NCH):
                        for k in range(KT):
                            nc.vector.bn_stats(cstats[k][:, cc, :],
                                               cA_s[:, k, cc, :])
                if ch == 5:
                    # k-outer so each k's aggregate chain starts as soon as
                    # its four cB chunks are done (pipelines with bn tail)
                    for k in range(KT):
                        for cc in range(NCH):
                            nc.vector.bn_stats(cstats[k][:, NCH + cc, :],
                                               cB_s[:, k, cc, :])
                        mv = s1.tile([P, 2], F32, tag="s2")
                        nc.vector.bn_aggr(mv[:], cstats[k][:, :, :])
                        mh = s1.tile([P, 1], F16, tag="s1h")
                        nc.vector.tensor_copy(mh[:], mv[:, 0:1])
                        cmean_h.append(mh)
                        vp_ = s1.tile([P, 1], F32, tag="s1")
                        nc.vector.tensor_scalar(vp_[:], mv[:, 1:2],
                                                HW / (HW - 1.0), EPS,
                                                op0=ALU.mult, op1=ALU.add)
                        c_varp.append(vp_)
                        st = s1.tile([P, 1], F32, tag="s1")
                        nc.scalar.activation(st[:], vp_[:], AF.Sqrt)
                        c_std.append(st)
                    for k in range(KT):
                        r = s1.tile([P, 1], F32, tag="s1")
                        nc.vector.reciprocal(r[:], c_std[k][:])
                        crstd.append(r)
                        nc.vector.tensor_scalar_mul(
                            B_rs[:, k, :], bmat_sb[:, k, :], r[:])

            # ---------- FB bias fixup mms + FB conv (raw; style-side
            # scale/bias applied later on vector once srstd lands) -------
            ubias = []
            for j in range(KT):
                psb = mmps.tile([P, 1], F32, tag="ups", name=f"ups{j}",
                                bufs=2)
                for k in range(KT):
                    nc.tensor.matmul(
                        psb[:], B_rs[:, k, j * P:(j + 1) * P], cmean_h[k][:],
                        start=(k == 0), stop=(k == KT - 1))
                bp_ = s1.tile([P, 1], F32, tag="s1")
                nc.vector.tensor_sub(bp_[:], pb_sb[:, KT + j:KT + j + 1],
                                     psb[:])
                ubias.append(bp_)
            for ch in range(NCH):
                for j in range(KT):
                    ps = stps.tile([P, 512], F32, tag="stps")
                    for k in range(KT):
                        nc.tensor.matmul(
                            ps[:], B_rs[:, k, j * P:(j + 1) * P],
                            cA_s[:, k, ch, :],
                            start=(k == 0), stop=(k == KT - 1))
                    nc.scalar.activation(
                        F_raw[j][:, ch * 512:(ch + 1) * 512], ps[:],
                        AF.Copy)
                if ch == 2:
                    # style stats tail on vector; sqrt on scalar after acts
                    for sch in range(4, MCH):
                        for k in range(KT):
                            nc.vector.bn_stats(sstats[k][:, sch, :],
                                               style_sb[:, sch, k, :])
                    for k in range(KT):
                        mv = s1.tile([P, 2], F32, tag="s2")
                        nc.vector.bn_aggr(mv[:], sstats[k][:, :, :])
                        vp_ = s1.tile([P, 1], F32, tag="s1")
                        nc.vector.tensor_scalar(vp_[:], mv[:, 1:2],
                                                HW / (HW - 1.0), EPS,
                                                op0=ALU.mult, op1=ALU.add)
                        s_varp.append(vp_)
                    for k in range(KT):
                        st = s1.tile([P, 1], F32, tag="s1")
                        nc.scalar.activation(st[:], s_varp[k][:], AF.Sqrt)
                        s_std.append(st)

            for k in range(KT):
                r = s1.tile([P, 1], F32, tag="s1")
                nc.vector.reciprocal(r[:], s_std[k][:])
                srstd.append(r)
            bias_t = []
            for j in range(KT):
                bt = s1.tile([P, 1], F32, tag="s1")
                nc.vector.tensor_mul(bt[:], ubias[j][:], srstd[j][:])
                bias_t.append(bt)
            # FB = F_raw * srstd + bias  (vector, ch0 first: gates scores)
            for ch in range(NCH):
                for j in range(KT):
                    nc.vector.tensor_scalar(
                        F_sb[j][:, ch * 512:(ch + 1) * 512],
                        F_raw[j][:, ch * 512:(ch + 1) * 512],
                        srstd[j][:], bias_t[j][:],
                        op0=ALU.mult, op1=ALU.add)

        # ---------- phase B: attention ----------
        with tc.tile_pool(name="expp", bufs=2) as expp, \
             tc.tile_pool(name="vp", bufs=2) as vp, \
             tc.tile_pool(name="styp", bufs=3) as styp, \
             tc.tile_pool(name="outp", bufs=3) as outp:
            for ch in range(NCH):
                exp_t = expp.tile([P, MT, 512], BF16, tag="exp")
                V = vp.tile([P, 512], F32, tag="V")
                V_bf = vp.tile([P, 512], BF16, tag="Vbf")
                rden = vp.tile([P, 512], F32, tag="rden")
                for mt in range(MT):
                    ps = stps.tile([P, 512], F32, tag="stps")
                    for k in range(KT):
                        nc.tensor.matmul(
                            ps[:],
                            style_sb[:, mt // 4, k,
                                     (mt % 4) * P:(mt % 4 + 1) * P],
                            F_sb[k][:, ch * 512:(ch + 1) * 512],
                            start=(k == 0), stop=(k == KT - 1))
                    nc.scalar.activation(exp_t[:, mt, :], ps[:], AF.Exp)
                    if mt == 0:
                        nc.vector.tensor_copy(V[:], exp_t[:, 0, :])
                    elif mt == MT - 1:
                        nc.vector.tensor_add(V_bf[:], V[:], exp_t[:, mt, :])
                    else:
                        nc.vector.tensor_add(V[:], V[:], exp_t[:, mt, :])

                apsums = []
                for j in range(KT):
                    ps = mmps.tile([P, 512], F32, tag="mmps")
                    for mt in range(MT):
                        nc.tensor.matmul(
                            ps[:], Ht_sb[:, mt, j * P:(j + 1) * P],
                            exp_t[:, mt, :],
                            start=(mt == 0), stop=(mt == MT - 1))
                    apsums.append(ps)
                    if j == 0:
                        dps = mmps.tile([P, 512], F32, tag="mmps")
                        nc.tensor.matmul(dps[:], ones_bf[:], V_bf[:],
                                         start=True, stop=True)
                        nc.vector.reciprocal(rden[:], dps[:])
                    s_t = styp.tile([P, 512], F32, tag="sty")
                    nc.vector.tensor_mul(s_t[:], ps[:], rden[:])
                    o_t = outp.tile([P, 512], F32, tag="outsb")
                    nc.scalar.activation(o_t[:], s_t[:], AF.Identity,
                                         bias=pb_sb[:, j:j + 1])
                    nc.sync.dma_start(
                        out[j * P:(j + 1) * P, ch * 512:(ch + 1) * 512],
                        o_t[:])


def _kmajor(x, cols):
    """[KT*128, cols] -> [128, KT*cols] with column block k = rows k*128.."""
    return np.ascontiguousarray(
        np.asarray(x).reshape(KT, P, cols).transpose(1, 0, 2)
        .reshape(P, KT * cols), dtype=np.float32)


_NC_CACHE = None


def _get_nc():
    global _NC_CACHE
    if _NC_CACHE is None:
        _NC_CACHE = build_nc()
    return _NC_CACHE


def make_in_maps(content, style, f_w, f_b, g_w, g_b, h_w, h_b, out_w, out_b):
    b, Cc, H, W = content.shape
    hw = H * W
    cf = np.ascontiguousarray(content.reshape(b, Cc, hw), dtype=np.float32)
    sf = np.ascontiguousarray(style.reshape(b, Cc, hw), dtype=np.float32)
    ob2 = (np.asarray(out_b, np.float64)
           + np.asarray(out_w, np.float64) @ np.asarray(h_b, np.float64))
    walpha = (np.asarray(g_w, np.float64).T @ np.asarray(f_b, np.float64))
    pbias = np.concatenate([
        ob2.astype(np.float32).reshape(KT, P).T,
        walpha.astype(np.float32).reshape(KT, P).T], axis=1)
    hw2 = np.asarray(out_w, np.float64) @ np.asarray(h_w, np.float64)
    bmat = (np.asarray(f_w, np.float64).T @ np.asarray(g_w, np.float64))
    wT = {
        "hw_bf": _kmajor(hw2.T.astype(np.float32), C).astype(np.float16),
        "bmat": _kmajor(bmat.astype(np.float32), C).astype(np.float16),
        "pbias": np.ascontiguousarray(pbias, dtype=np.float32),
    }
    in_maps = []
    for core in range(8):
        bi, hi = core // 2, core % 2
        in_maps.append({
            "cA": _kmajor(cf[bi][:, hi * NLOC:(hi + 1) * NLOC],
                          NLOC).astype(np.float16),
            "cB_bf": _kmajor(cf[bi][:, (1 - hi) * NLOC:(2 - hi) * NLOC],
                             NLOC).astype(np.float16),
            "style_bf": np.concatenate(
                [_kmajor(sf[bi][:, ch * 512:(ch + 1) * 512], 512)
                 for ch in range(MCH)], axis=1).astype(np.float16),
            **wT,
        })
    return in_maps


def kernel(content, style, f_w, f_b, g_w, g_b, h_w, h_b, out_w, out_b):
    from concourse.bass_utils import run_bass_kernel_spmd

    global _LAST_IN_MAPS
    in_maps = make_in_maps(content, style, f_w, f_b, g_w, g_b, h_w, h_b,
                           out_w, out_b)
    _LAST_IN_MAPS = in_maps
    b, Cc, H, W = content.shape
    hw = H * W
    nc = _get_nc()
    res = run_bass_kernel_spmd(nc, in_maps, core_ids=list(range(8)))
    outf = np.empty((b, Cc, hw), dtype=np.float32)
    for core in range(8):
        bi, hi = core // 2, core % 2
        outf[bi][:, hi * NLOC:(hi + 1) * NLOC] = res.results[core]["out"]
    return outf.reshape(b, Cc, H, W)
